# revision 1
# baseline (speedup 1.0000x reference)
"""Trainium2 Bass kernel for nn_Attention_54013508715307.

Attention with a Klein-bottle geometric bias, data-parallel over batch:
each of the 8 NeuronCores processes one batch element end-to-end (no
collectives).

Key device-side design points:
 - The geometric bias matrix G = exp(-klein_dist^2/sigma^2) is built on the
   PE as max of two rank-121 matmuls: exp(-circ(t)^2) is expanded in a
   truncated Fourier series (K=6 harmonics per axis); the per-point Fourier
   features are computed on the host from klein_coords and shipped as
   [121, 1024] matrices.  max error vs exact: ~2e-3 in bf16.
 - Scores are computed transposed (ST[m, n] = k_m . q_n, keys on
   partitions) so softmax never needs a transpose: the denominator comes
   free from an appended ones-column in the attn @ v matmul, and the
   attn @ v matmul consumes exp(ST) tiles directly as stationary operands.
 - All matmuls run in bf16 (f32 PSUM accumulation).  End-to-end rel err vs
   the f32 reference: ~7e-3.
 - Engine split for the O(N^2) elementwise work per head:
   GpSimd: bias = G * gate_bcast (bf16), DVE: T = bias + ST_psum,
   ACT: exp, PE: everything matmul.
"""

import math

import numpy as np
import ml_dtypes

bf16 = ml_dtypes.bfloat16
TWO_PI = 2.0 * np.pi
PI = np.pi

H, DH = 8, 64
B, N, D = 8, 1025, 512
NPATCH = 1024
KF = 6                    # Fourier harmonics per axis
NF = 2 * KF - 1           # 11 per-axis features (cos k=0..5, sin k=1..5)
RANK = NF * NF            # 121

# token tiles for the key (m) axis, aligned so patch tiles match G rows
MT = [(0, 1)] + [(1 + 128 * i, 128) for i in range(8)]
# token tiles for query/output rows
NT = [(128 * i, 128) for i in range(8)] + [(1024, 1)]
# moving-operand chunks along the 1025-wide token axis
CH = [(0, 512), (512, 512), (1024, 1)]

_CACHE = {}


def _fourier_coeffs(sigma):
    n = 1 << 16
    t = np.arange(n) * (TWO_PI / n)
    circ = PI - np.abs(np.abs(np.mod(t, TWO_PI)) - PI)
    f = np.exp(-circ * circ / (sigma * sigma))
    F = np.fft.rfft(f) / n
    a = np.zeros(KF)
    a[0] = F[0].real
    a[1:] = 2.0 * F[1:KF].real
    return a


def _features(v, coef=None, sin_sign=1.0):
    # [len(v), NF]: cos(k v) for k=0..KF-1 then sin(k v) for k=1..KF-1
    ks = np.arange(KF)
    U = np.concatenate(
        [np.cos(np.outer(v, ks)), np.sin(np.outer(v, ks[1:]))], axis=1
    )
    if coef is not None:
        U = U * np.concatenate([coef, coef[1:] * sin_sign])
    return U


def _khatri_rao(A, Bm):
    return (A[:, :, None] * Bm[:, None, :]).reshape(A.shape[0], -1)


def _enable_ldw_opt():
    # Dedupe consecutive LDWEIGHTS of identical stationary operands: flip the
    # hardcoded --enable-ldw-opt=false in walrus invocations.
    import concourse.bass_utils as bu

    if getattr(bu, "_ldw_opt_patched", False):
        return
    orig = bu.run_command

    def patched(argv, **kw):
        argv = ["--enable-ldw-opt=true" if a == "--enable-ldw-opt=false" else a
                for a in argv]
        return orig(argv, **kw)

    bu.run_command = patched
    bu._ldw_opt_patched = True


def _build_program(bg_val):
    import bass_rust
    import concourse.bass as bass
    import concourse.mybir as mybir
    import concourse.tile as tile


    def _drain_and_barrier_split(self, tick_clock, wait_clock):
        # Walrus in this container rejects more than a couple of waits on
        # the kernel-tail Drain; emit one sync-engine nop per waited proc.
        gc = list(tick_clock.global_clock)
        n = len(gc)
        for i, t in enumerate(gc):
            if t == 0:
                continue
            vc = [0] * n
            vc[i] = t
            nop = self.nc.sync.nop()
            wait_clock.add_sem_waits(
                nop.ins, tile.ScopedClock({None: bass_rust.VectorClock(vc)})
            )
        self.nc.sync.drain()
        self.nc.all_engine_barrier()
        popped = self.nc._tile_sem_poison_stack.pop()
        assert popped is self._sem_poison
        self.nc.clear_and_free_semaphores(list(self.sems.allocated().values()))
        self.nc.all_engine_barrier()

    tile.TileContext._drain_and_barrier = _drain_and_barrier_split

    from concourse.masks import make_identity

    dt = mybir.dt
    BF = dt.bfloat16
    F32 = dt.float32
    Alu = mybir.AluOpType
    Act = mybir.ActivationFunctionType

    nc = bass.Bass()
    x_d = nc.declare_dram_parameter("x", [N, D], BF, isOutput=False)
    wq_d = nc.declare_dram_parameter("wq", [D, 512], BF, isOutput=False)
    wk_d = nc.declare_dram_parameter("wk", [D, 512], BF, isOutput=False)
    wv_d = nc.declare_dram_parameter("wv", [D, 512], BF, isOutput=False)
    wo_d = nc.declare_dram_parameter("wo", [512, D], BF, isOutput=False)
    wgx_d = nc.declare_dram_parameter("wgx", [D, H], BF, isOutput=False)
    bo_d = nc.declare_dram_parameter("bo", [D], F32, isOutput=False)
    pt_d = nc.declare_dram_parameter("pt", [RANK, NPATCH], BF, isOutput=False)
    qt_d = nc.declare_dram_parameter("qt", [RANK, NPATCH], BF, isOutput=False)
    qw_d = nc.declare_dram_parameter("qw", [RANK, NPATCH], BF, isOutput=False)
    out_d = nc.declare_dram_parameter("out", [N, D], F32, isOutput=True)

    def bcast_rows(src_ap, nrows):
        # replicate a [1, F] AP across nrows partitions (DMA source)
        return bass.AP(
            tensor=src_ap.tensor,
            offset=src_ap.offset,
            ap=[[0, nrows]] + list(src_ap.ap[-1:]),
        )

    with tile.TileContext(nc) as tc:
        with tc.tile_pool(name="sing", bufs=1) as sing, \
             tc.tile_pool(name="sb", bufs=1) as sb, \
             tc.tile_pool(name="att", bufs=2) as att, \
             tc.tile_pool(name="wrk", bufs=3) as wrk, \
             tc.tile_pool(name="dramp", bufs=1, space="DRAM") as dramp:

            ident = sing.tile([128, 128], BF, tag="ident", name="ident")
            make_identity(nc, ident)

            bo_bc = sing.tile([128, 512], F32, tag="bo", name="bo")
            nc.scalar.dma_start(out=bo_bc, in_=bcast_rows(bo_d[None, :], 128))

            gate_bf = sing.tile([8, 1025], BF, tag="gate", name="gate")
            gsc = dramp.tile([8, 1025], BF, tag="gsc", name="gsc")

            # ---- phase A/B/C pools that release early -------------------
            with tc.tile_pool(name="pw", bufs=1) as pw, \
                 tc.tile_pool(name="pq", bufs=1) as pq:

                # ---- xT = x.T via DMA transpose -------------------------
                xT = [sb.tile([128, 1025], BF, tag=f"xT{j}", name=f"xT{j}") for j in range(4)]
                for j in range(4):
                    nc.sync.dma_start_transpose(
                        xT[j][:, 0:1024], x_d[0:1024, j * 128:(j + 1) * 128]
                    )
                    nc.sync.dma_start(
                        out=xT[j][:, 1024:1025],
                        in_=x_d[1024:1025, j * 128:(j + 1) * 128].rearrange("a b -> b a"),
                    )

                pt_sb = pq.tile([RANK, NPATCH], BF, tag="pt", name="pt")
                qt_sb = pq.tile([RANK, NPATCH], BF, tag="qt", name="qt")
                qw_sb = pq.tile([RANK, NPATCH], BF, tag="qw", name="qw")
                nc.scalar.dma_start(out=pt_sb, in_=pt_d[:, :])
                nc.scalar.dma_start(out=qt_sb, in_=qt_d[:, :])
                nc.scalar.dma_start(out=qw_sb, in_=qw_d[:, :])

                # ---- G = max(P Qt^T, P Qw^T)  (rank-121 Fourier) --------
                G = [sb.tile([128, NPATCH], BF, tag=f"G{i}", name=f"G{i}") for i in range(8)]
                ppD = tc.tile_pool(name="ppD", bufs=2, space="PSUM")
                with ppD as pp:
                  for i in range(8):
                    ga = pp.tile([128, NPATCH], F32, tag="ga", name="ga")
                    gb = pp.tile([128, NPATCH], F32, tag="gb", name="gb")
                    for (c0, cw) in ((0, 512), (512, 512)):
                        nc.tensor.matmul(
                            ga[:, c0:c0 + cw],
                            lhsT=pt_sb[:, i * 128:(i + 1) * 128],
                            rhs=qt_sb[:, c0:c0 + cw],
                            start=True, stop=True,
                        )
                        nc.tensor.matmul(
                            gb[:, c0:c0 + cw],
                            lhsT=pt_sb[:, i * 128:(i + 1) * 128],
                            rhs=qw_sb[:, c0:c0 + cw],
                            start=True, stop=True,
                        )
                    gtmp = wrk.tile([128, NPATCH], F32, tag="gtmp", name="gtmp", bufs=2)
                    nc.scalar.copy(gtmp, ga)
                    nc.vector.tensor_tensor(G[i], gtmp, gb, Alu.max)

                wq_sb, wk_sb, wv_sb, wgx_sb = [], [], [], []
                for k in range(4):
                    for lst, dram, w in ((wq_sb, wq_d, 512), (wk_sb, wk_d, 512),
                                         (wv_sb, wv_d, 512), (wgx_sb, wgx_d, H)):
                        t = pw.tile([128, w], BF, tag=f"w{len(lst)}_{id(dram)%97}", name=f"w{len(lst)}_{id(dram)%97}")
                        eng = nc.sync if lst is wq_sb else (
                            nc.scalar if lst is wk_sb else nc.gpsimd)
                        eng.dma_start(out=t, in_=dram[k * 128:(k + 1) * 128, :])
                        lst.append(t)
                wo_sb = []
                for k in range(4):
                    t = sb.tile([128, 512], BF, tag=f"wo{k}", name=f"wo{k}")
                    nc.gpsimd.dma_start(out=t, in_=wo_d[k * 128:(k + 1) * 128, :])
                    wo_sb.append(t)
                # ---- projections ---------------------------------------
                qT = [sb.tile([128, 1025], BF, tag=f"qT{j}", name=f"qT{j}") for j in range(4)]
                kTt = [sb.tile([128, 1025], BF, tag=f"kT{j}", name=f"kT{j}") for j in range(4)]
                ppB = tc.tile_pool(name="ppB", bufs=2, space="PSUM")
                with ppB as pp:
                 # gate logits first (consumed by every head's bias)
                 ps = pp.tile([128, 1025], F32, tag="big", name="big")
                 for (c0, cw) in CH:
                    for k in range(4):
                        nc.tensor.matmul(
                            ps[:8, c0:c0 + cw],
                            lhsT=wgx_sb[k],
                            rhs=xT[k][:, c0:c0 + cw],
                            start=(k == 0), stop=(k == 3),
                        )
                 nc.scalar.activation(gate_bf, ps[:8], Act.Sigmoid, bias=float(bg_val))
                 nc.sync.dma_start(out=gsc[:, 0:1024], in_=gate_bf[:, 1:1025])

                 for j in range(4):
                    for dst, wsb in ((qT, wq_sb), (kTt, wk_sb)):
                        ps = pp.tile([128, 1025], F32, tag="big", name="big")
                        for (c0, cw) in CH:
                            for k in range(4):
                                nc.tensor.matmul(
                                    ps[:, c0:c0 + cw],
                                    lhsT=wsb[k][:, j * 128:(j + 1) * 128],
                                    rhs=xT[k][:, c0:c0 + cw],
                                    start=(k == 0), stop=(k == 3),
                                )
                        nc.scalar.copy(dst[j], ps)

                 vp = [sb.tile([128, 8, 65], BF, tag=f"vp{i}", name=f"vp{i}") for i in range(9)]
                 for mi, (m0, mw) in enumerate(MT):
                    ps = pp.tile([128, 512], F32, tag="mid", name="mid")
                    for k in range(4):
                        nc.tensor.matmul(
                            ps[:mw],
                            lhsT=xT[k][:, m0:m0 + mw],
                            rhs=wv_sb[k],
                            start=(k == 0), stop=(k == 3),
                        )
                    nc.scalar.copy(
                        vp[mi][:mw, :, 0:64],
                        ps[:mw].rearrange("p (h c) -> p h c", h=8),
                    )
                    nc.gpsimd.memset(vp[mi][:mw, :, 64:65], 1.0)


            # ---- attention ---------------------------------------------
            outAll = [sb.tile([128, 512], BF, tag=f"oa{i}", name=f"oa{i}")
                      for i in range(9)]
            ppE = tc.tile_pool(name="ppE", bufs=2, space="PSUM")
            with ppE as pp:
             for h in range(8):
                 jr, pr = h // 2, 64 * (h % 2)
                 gabc = att.tile([128, 1024], BF, tag="gabc", name="gabc",
                                 bufs=3)
                 nc.sync.dma_start(
                     out=gabc, in_=bcast_rows(gsc[h:h + 1, 0:1024], 128)
                 )

                 eT = []
                 for mi, (m0, mw) in enumerate(MT):
                     eT.append(att.tile([mw, 1026], BF, tag=f"e{mi}", name=f"e{mi}"))
                 for mi, (m0, mw) in enumerate(MT):
                     ps = pp.tile([128, 1025], F32, tag="big", name="big")
                     for (p0, t0, cw) in ((0, 1, 512), (512, 513, 512),
                                          (1024, 0, 1)):
                         nc.tensor.matmul(
                             ps[:mw, p0:p0 + cw],
                             lhsT=kTt[jr][pr:pr + 64, m0:m0 + mw],
                             rhs=qT[jr][pr:pr + 64, t0:t0 + cw],
                             start=True, stop=True,
                         )
                     if mi == 0:
                         nc.scalar.activation(eT[0][:, 0:1025], ps[:1], Act.Exp)
                     else:
                         ht = wrk.tile([128, 1026], BF, tag="ht", name="ht", bufs=6)
                         nc.gpsimd.memset(ht[:, 1024:1025], 0.0)
                         if (h + mi) % 8 in (1, 5):
                             nc.vector.tensor_tensor(
                                 ht[:, 0:1024], G[mi - 1], gabc, Alu.mult
                             )
                         else:
                             nc.gpsimd.tensor_tensor(
                                 ht[:, 0:1024], G[mi - 1], gabc, Alu.mult
                             )
                         tt_ = wrk.tile([128, 1026], BF, tag="tt", name="tt", bufs=4)
                         nc.vector.tensor_tensor(
                             tt_[:, 0:1025], ht[:, 0:1025], ps[:mw], Alu.add
                         )
                         nc.scalar.activation(eT[mi], tt_[:mw], Act.Exp)

                 for ni, (p0, nw) in enumerate(NT):
                     vo = pp.tile([128, 65], F32, tag="vo", name="vo")
                     for mi, (m0, mw) in enumerate(MT):
                         nc.tensor.matmul(
                             vo[:nw],
                             lhsT=eT[mi][:mw, p0:p0 + nw],
                             rhs=vp[mi][:mw, h, :],
                             start=(mi == 0), stop=(mi == 8),
                         )
                     rcp = wrk.tile([128, 1], F32, tag="rcp", name="rcp")
                     nc.vector.reciprocal(rcp[:nw], vo[:nw, 64:65])
                     nc.vector.tensor_scalar(
                         outAll[ni][:nw, h * 64:(h + 1) * 64],
                         vo[:nw, 0:64], rcp[:nw], None, Alu.mult,
                     )

            # ---- output projection -------------------------------------
            oT = [sb.tile([128, 1025], BF, tag=f"oT{j}", name=f"oT{j}")
                  for j in range(4)]
            ppF = tc.tile_pool(name="ppF", bufs=2, space="PSUM")
            with ppF as pp:
             for ni, (p0, nw) in enumerate(NT):
                for j in range(4):
                    ps = pp.tile([128, 128], BF, tag="tp", name="tp")
                    nc.tensor.transpose(
                        ps[:128, :nw],
                        outAll[ni][:nw, j * 128:(j + 1) * 128],
                        ident[:nw, :nw],
                    )
                    nc.scalar.copy(oT[j][:, p0:p0 + nw], ps[:, :nw])
             for ni, (p0, nw) in enumerate(NT):
                ps = pp.tile([128, 512], F32, tag="mid", name="mid")
                for j in range(4):
                    nc.tensor.matmul(
                        ps[:nw],
                        lhsT=oT[j][:, p0:p0 + nw],
                        rhs=wo_sb[j],
                        start=(j == 0), stop=(j == 3),
                    )
                y = wrk.tile([128, 512], F32, tag="y", name="y")
                nc.vector.tensor_tensor(y[:nw], ps[:nw], bo_bc[:nw], Alu.add)
                if ni < 8:
                    nc.sync.dma_start(out=out_d[1 + p0:1 + p0 + nw, :], in_=y[:nw])
                else:
                    nc.sync.dma_start(out=out_d[0:1, :], in_=y[:1])

    return nc


_MAXW = {"Matmult": 1}  # per-opcode max sync waits; walrus default cap below
_MAXW_DEFAULT = 1


def _split_waits_json(raw):
    """Walrus rejects instructions with more than a couple of sem waits.
    Move excess on_wait entries onto NoOp instructions inserted just before
    the offending instruction on the same engine (semantically identical:
    the engine stalls at the nop first)."""
    import orjson

    bir = orjson.loads(raw)
    uid = [0]
    for f in bir["functions"]:
        for blk in f["blocks"]:
            insts = blk["instructions"]
            out = []
            for ins in insts:
                si = ins.get("sync_info")
                waits = si.get("on_wait", []) if si else []
                maxw = _MAXW.get(ins["opcode"], _MAXW_DEFAULT)
                if len(waits) > maxw:
                    keep = waits[-maxw:]
                    extra = waits[:-maxw]
                    nopw = _MAXW.get("NoOp", _MAXW_DEFAULT)
                    for c0 in range(0, len(extra), nopw):
                        chunk = extra[c0:c0 + nopw]
                        uid[0] += 1
                        out.append({
                            "debug": ins.get("debug", 0),
                            "engine": ins["engine"],
                            "ins": [],
                            "name": f"{ins['name']}_ws{uid[0]}",
                            "opcode": "NoOp",
                            "outs": [],
                            "sync_info": {"on_update": [], "on_wait": chunk},
                        })
                    si["on_wait"] = keep
                out.append(ins)
            blk["instructions"] = out
    return orjson.dumps(bir)


def _get_program(bg_val):
    key = ("prog", float(bg_val))
    if key not in _CACHE:
        nc = _build_program(bg_val)
        patched = _split_waits_json(nc.to_json_bytes())
        nc.to_json_bytes = lambda: patched
        _CACHE[key] = nc
    return _CACHE[key]


def kernel(x, klein_coords, Wqkv, Wg, bg, Wo, bo, alpha, sigma, **_ignored):
    from concourse.bass_utils import run_bass_kernel_spmd

    x = np.asarray(x, np.float32)
    klein_coords = np.asarray(klein_coords, np.float32)
    Wqkv = np.asarray(Wqkv, np.float32)
    Wg = np.asarray(Wg, np.float32)
    bg_val = float(np.asarray(bg).reshape(-1)[0])
    Wo = np.asarray(Wo, np.float32)
    bo = np.asarray(bo, np.float32).reshape(D)
    alpha_v = float(np.asarray(alpha))
    sigma_v = float(np.asarray(sigma))

    scale = DH ** -0.5
    Wq = Wqkv[:, :512]
    Wk = Wqkv[:, 512:1024] * scale   # fold softmax scale into k projection
    Wv = Wqkv[:, 1024:]
    WgBD = np.zeros((512, H), np.float32)
    for h in range(H):
        WgBD[h * 64:(h + 1) * 64, h] = Wg[:, 0]
    preGW = Wq @ WgBD                # gate logits = x @ preGW + bg

    a = _fourier_coeffs(sigma_v)
    ks = np.arange(KF)
    a_tw = a * ((-1.0) ** ks)

    nc = _get_program(bg_val)

    in_maps = []
    for b in range(B):
        cx = klein_coords[b, :, 0]
        cy = klein_coords[b, :, 1]
        P = _khatri_rao(_features(cx), _features(cy))
        Qt = alpha_v * _khatri_rao(_features(cx, a), _features(cy, a))
        Qw = alpha_v * _khatri_rao(_features(cx, a_tw), _features(cy, a, -1.0))
        in_maps.append({
            "x": x[b].astype(bf16),
            "wq": Wq.astype(bf16),
            "wk": Wk.astype(bf16),
            "wv": Wv.astype(bf16),
            "wo": Wo.astype(bf16),
            "wgx": preGW.astype(bf16),
            "bo": bo,
            "pt": np.ascontiguousarray(P.T).astype(bf16),
            "qt": np.ascontiguousarray(Qt.T).astype(bf16),
            "qw": np.ascontiguousarray(Qw.T).astype(bf16),
        })

    res = run_bass_kernel_spmd(nc, in_maps, core_ids=list(range(8)))
    _CACHE["last_res"] = res
    out = np.stack([r["out"] for r in res.results], axis=0)
    return out.astype(np.float32)


if __name__ == "__main__":
    rng = np.random.default_rng(0)
    inputs = {
        "x": rng.standard_normal((B, N, D), dtype=np.float32),
        "klein_coords": rng.uniform(0, TWO_PI, (B, N - 1, 2)).astype(np.float32),
        "Wqkv": (rng.standard_normal((D, 3 * 512), dtype=np.float32) * D ** -0.5),
        "Wg": (rng.standard_normal((DH, 1), dtype=np.float32) * DH ** -0.5),
        "bg": np.zeros((1,), np.float32),
        "Wo": (rng.standard_normal((512, D), dtype=np.float32) * 512 ** -0.5),
        "bo": np.zeros((D,), np.float32),
        "alpha": np.array(1.0, np.float32),
        "sigma": np.array(1.0, np.float32),
    }
    out = kernel(**inputs)
    print("out", out.shape, out.dtype, np.abs(out).mean())

